# revision 30
# baseline (speedup 1.0000x reference)
"""Distributed cross-entropy-over-feature-bank kernel for 8 trn2 NeuronCores.

Problem: loss = masked-mean NLL of log_softmax(inputs @ features.T / TEMP)
  inputs   [256, 2048] f32 (L2-normalized rows)
  targets  [256] int (1-based; 0 -> invalid; 5554 -> ignore class 1023)
  features [16384, 2048] f32 (L2-normalized rows)

Sharding: feature bank split row-wise, 2048 rows per core. Each core computes
its partial logits tile [256, 2048] = inputs @ shard.T on TensorE and reduces
it to per-batch-row sums of exp(logits/TEMP) (exp + row-sum fused in one
ScalarE activation with accum_out). Host combines the 8 partial sums, adds the
target-logit term (a 256-row gather/dot, ~0.006% of the FLOPs, done in f64)
and the valid-row masking to produce the scalar loss.

log-softmax without max-subtraction is safe: logits = cosine/0.05 lie in
[-20, 20], so sum(exp) <= 16384 * e^20 ~ 8e12, far below f32 overflow.

Both operands are pre-swizzled on host into exact SBUF images so every DMA is
a contiguous 2D slab. Matmul dtype selectable via KERNEL_MM_DTYPE:
  fp8  - e4m3 (inputs scaled by 16 to use the fp8 range; rescaled in the exp)
         with DoubleRow: 2 MACs/cell/cycle, quarter DMA bytes. Loss error
         stays ~1e-4: the target-logit term is exact (host f64), so fp8 noise
         only enters through log(sum exp), which compresses it.
  bf16 - 1 cycle/row, half DMA bytes, loss error ~1e-6.
  f32r - full f32 bytes, reduced-precision multiplies, 1 cycle/row.
"""

import os
from contextlib import ExitStack

import ml_dtypes  # noqa: F401  (bf16/fp8 numpy dtypes via mybir.dt.np)
import numpy as np

import concourse.bass as bass  # noqa: F401
import concourse.mybir as mybir
import concourse.tile as tile
from concourse import bacc
from concourse.bass import ts
from concourse.bass_utils import run_bass_kernel_spmd

NCORES = 8
B = 256           # batch rows
D = 2048          # feature dim (matmul contraction)
S = 16384         # feature-bank rows
SH = S // NCORES  # bank rows per core
TEMP = 0.05
SPECIAL_LABEL = 5554
IGNORE = 1023     # SOURCE_CLASSES - 1

KT = D // 128     # 16 contraction k-tiles
W = 512           # output-column group width (one PSUM bank)
G = SH // W       # 4 column groups per core
NM = B // 128     # 2 batch-row tiles
FP8_SCALE = 16.0  # use the e4m3 range; folded back in the exp scale

# k-tiles per DMA chunk, per column group. Group 0 leads with small chunks so
# the first matmul starts as soon as ~3 small slabs have landed; later groups
# use one big slab each (every DMA pays a serialized HWDGE descriptor-gen
# slot, so fewer+bigger wins once the pipe is primed). All chunks even-sized
# so fp8 DoubleRow k-pairs never straddle a chunk boundary.
# Mid-stream PE stalls are irrelevant (PE work is ~half the DMA time), so
# minimize DMA count (each costs a serialized HWDGE descriptor-gen slot) and
# taper only the LAST group so little PE work remains after the last byte.
CHUNK_PLANS = [[KT], [KT], [KT], [8, 4, 4]]

MM_DTYPE = os.environ.get("KERNEL_MM_DTYPE", "fp8")  # "fp8"|"bf16"|"f32r"|"f32"

_nc_cache = {}


def _io_dtype(tag):
    return {"fp8": mybir.dt.float8e4, "bf16": mybir.dt.bfloat16,
            "f32r": mybir.dt.float32r, "f32": mybir.dt.float32}[tag]


def _build_nc(tag):
    io_dt = _io_dtype(tag)

    exp_scale = (1.0 / TEMP) / (FP8_SCALE * FP8_SCALE if tag == "fp8" else 1.0)

    nc = bacc.Bacc("TRN2", target_bir_lowering=False, debug=False,
                   num_devices=NCORES)
    xT = nc.dram_tensor("xT", [128, KT * B], io_dt, kind="ExternalInput").ap()
    fT = nc.dram_tensor("fT", [128, KT * SH], io_dt,
                        kind="ExternalInput").ap()
    out = nc.dram_tensor("out", [128, NM * G], mybir.dt.float32,
                         kind="ExternalOutput").ap()

    nchunks = sum(len(p) for p in CHUNK_PLANS)

    with tile.TileContext(nc) as tc, ExitStack() as ctx:
        cpool = ctx.enter_context(tc.tile_pool(name="const", bufs=1))
        # one slot per chunk (unique tags, bufs=1): a DMACopy can encode at
        # most ONE sync-wait, so slot reuse (which would add WAR+WAW waits on
        # the DMA) is avoided.
        del nchunks
        fpool = ctx.enter_context(tc.tile_pool(name="feat", bufs=1))
        epool = ctx.enter_context(tc.tile_pool(name="exp", bufs=3))
        pspool = ctx.enter_context(tc.tile_pool(name="ps", bufs=4, space="PSUM"))

        sums = cpool.tile([128, NM * G], mybir.dt.float32)

        xtile = cpool.tile([128, KT * B], io_dt)

        # DMA issue order = consumption order: all of x first (it gates the
        # first LDWEIGHTS), then the feature slabs group by group. Every DMA
        # pays a serialized HWDGE descriptor-gen slot (~625 ns) that delays
        # the stream end, so the count is kept minimal; mid-stream PE stalls
        # at slab boundaries are free (PE work is ~half the DMA time).
        nc.sync.dma_start(xtile[:], xT[:])
        chunk_of = {}      # (g, t) -> (tile, t_local)
        off = 0
        for g, plan in enumerate(CHUNK_PLANS):
            k0 = 0
            for ci, nk in enumerate(plan):
                fc = fpool.tile([128, nk * W], io_dt, tag=f"fc{g}_{ci}",
                                name=f"fc{g}_{ci}")
                nc.sync.dma_start(fc[:], fT[:, off:off + nk * W])
                for tl in range(nk):
                    chunk_of[(g, k0 + tl)] = (fc, tl)
                off += nk * W
                k0 += nk

        x3 = xtile[:].rearrange("p (t b) -> p t b", t=KT)

        for g in range(G):
            pss = [pspool.tile([128, W], mybir.dt.float32, tag="ps",
                               name=f"ps_{g}_{m}") for m in range(NM)]
            if tag == "fp8":
                for td in range(0, KT, 2):
                    fc, tl = chunk_of[(g, td)]
                    _, tl1 = chunk_of[(g, td + 1)]
                    assert tl1 == tl + 1, "k-pair straddles chunk"
                    c3 = fc[:].rearrange("p (t w) -> p t w", w=W)
                    rhs = c3[:, tl:tl + 2, :]
                    for m in range(NM):
                        nc.tensor.matmul(
                            pss[m][:], x3[:, td:td + 2, ts(m, 128)], rhs,
                            start=(td == 0), stop=(td == KT - 2),
                            perf_mode=mybir.MatmulPerfMode.DoubleRow,
                        )
            else:
                for t in range(KT):
                    fc, tl = chunk_of[(g, t)]
                    rhs = fc[:, ts(tl, W)]
                    for m in range(NM):
                        nc.tensor.matmul(
                            pss[m][:], x3[:, t, ts(m, 128)], rhs,
                            start=(t == 0), stop=(t == KT - 1),
                        )
            for m in range(NM):
                etile = epool.tile([128, W], mybir.dt.float32, name=f"e{g}{m}")
                nc.scalar.activation(
                    etile[:], pss[m][:],
                    mybir.ActivationFunctionType.Exp,
                    scale=exp_scale,
                    accum_out=sums[:, m * G + g: m * G + g + 1],
                )
        # per-group partial sums go out as-is; host reduces the G columns
        nc.sync.dma_start(out[:], sums[:])
    nc.compile()
    return nc


def _get_nc(tag):
    if tag not in _nc_cache:
        _nc_cache[tag] = _build_nc(tag)
    return _nc_cache[tag]


def _host_images(inputs, features, tag):
    """Pre-swizzle operands into per-core SBUF images (contiguous DMA slabs).

    xhost[p, t*B + b]            = inputs[b, t*128 + p]  (* scale)
    fhost_c[p, chunk-image cols] = features[c*SH + g*W + w, (k0+tl)*128 + p]
    """
    np_dt = mybir.dt.np(_io_dtype(tag))
    scale = FP8_SCALE if tag == "fp8" else 1.0

    xs = (inputs * scale) if scale != 1.0 else inputs
    xhost = np.ascontiguousarray(
        xs.T.reshape(KT, 128, B).transpose(1, 0, 2).reshape(128, KT * B)
    ).astype(np_dt)

    fs = (features * scale) if scale != 1.0 else features
    fhosts = []
    for c in range(NCORES):
        Fc = fs[c * SH:(c + 1) * SH]                      # [SH, D]
        I3 = Fc.reshape(G * W, KT, 128).transpose(2, 1, 0)  # [p, t, s]
        blocks = []
        for g, plan in enumerate(CHUNK_PLANS):
            k0 = 0
            for nk in plan:
                blocks.append(np.ascontiguousarray(
                    I3[:, k0:k0 + nk, g * W:(g + 1) * W]
                ).reshape(128, nk * W))
                k0 += nk
        fhosts.append(np.concatenate(blocks, axis=1).astype(np_dt))
    return xhost, fhosts


def kernel(inputs, targets, features, _collect=None):
    inputs = np.asarray(inputs)
    targets = np.asarray(targets)
    features = np.asarray(features)

    tag = MM_DTYPE
    xhost, fhosts = _host_images(inputs, features, tag)
    in_maps = [{"xT": xhost, "fT": fhosts[c]} for c in range(NCORES)]

    nc = _get_nc(tag)
    kwargs = dict(_collect or {})
    res = run_bass_kernel_spmd(nc, in_maps, core_ids=list(range(NCORES)),
                               **kwargs)
    if _collect is not None:
        _collect["results"] = res

    Ssum = np.zeros(B, np.float64)
    for c in range(NCORES):
        # out[p, m*G + g] = sum over group g's columns for batch row m*128+p
        o = np.asarray(res.results[c]["out"]).astype(np.float64)
        Ssum += o.reshape(128, NM, G).sum(axis=2).T.reshape(B)

    t = targets.astype(np.int64) - 1
    t = np.where(t == SPECIAL_LABEL, IGNORE, t)
    valid = (t >= 0) & (t != IGNORE)
    tcl = np.clip(t, 0, S - 1)
    g = (inputs.astype(np.float64) *
         features.astype(np.float64)[tcl]).sum(axis=1) / TEMP
    nll = np.log(Ssum) - g
    n_valid = int(valid.sum())
    loss = nll[valid].sum() / max(n_valid, 1)
    return np.asarray(loss, dtype=np.float32)
